# revision 31
# baseline (speedup 1.0000x reference)
"""Distributed GCN (3x GCNConv + FC) kernel for 8 Trainium2 NeuronCores. v2

Self-contained: kernel(**inputs) takes the full (unsharded) inputs and
returns the full [100000, 16] float32 output.

v2 strategy: destination nodes sharded across 8 cores; per core the
destination segments are packed into NGRP psum groups of 512 columns.  All
three layers' tables live in ONE slot space (rows = slots, chunk-major for
pipelined AllGather; bank = row%4 via the col===node (mod 4) constraint), so
a single static gather-index set and a single set of host-baked S matrices
(overlapping 64-col windows, dinv weights folded in) drive all 3
aggregations.  Aggregation: psum group zeroed by one matmul, then 64
accumulating matmuls (16 subtiles x 4 banks) per group per layer.
"""
import sys
for _p in ("/opt/trn_rl_repo", "/root/.axon_site/_ro/trn_rl_repo"):
    if _p not in sys.path:
        sys.path.append(_p)
import numpy as np
import concourse.bass as bass
import concourse.bacc as bacc
import concourse.tile as tile
import concourse.mybir as mybir
from concourse.bass_utils import run_bass_kernel_spmd

N_NODES = 100000
N_CORES = 8
P = 128
NBANK = 4
GCOLS = 512          # psum columns per group
NSUB = 16            # subtiles (windows) per group per bank
NCHUNK = 4           # AllGather chunks

F32 = mybir.dt.float32
BF16 = mybir.dt.bfloat16
I16 = mybir.dt.int16
AF = mybir.ActivationFunctionType
OP = mybir.AluOpType
NPBF = mybir.dt.np(BF16)   # numpy bfloat16 (ml_dtypes)
TW = 128                   # gather element width in bf16 elems (256B)
TBW = 64                   # tight table row width (bf16); elements are
                           # 256B strided windows covering 2 rows
DEBUG_DUMPS = False        # dump tab_sh tables as ExternalOutputs


# ---------------------------------------------------------------- packing --
def _assign_groups(cnt, res, tot, shard):
    """seg -> group. cnt [shard,4] bank counts, res [shard] node%4.
    Returns (grp, NGRP)."""
    CAPB = 2042          # per-(group,bank) edge cap (<= 2048 with margin)
    CAPR = 128           # per-(group,residue) seg cap (<= 128 cols)
    NGRP = max(int(np.ceil(cnt.sum(0).max() / 2040.0)),
               int(np.ceil(shard / 512.0)))
    while NGRP * GCOLS > 15872:   # keep 2*NSLOT < 32768 for int16 bankrows
        raise AssertionError("NGRP too large")
    attempt = 0
    while True:
        grp = np.empty(shard, dtype=np.int32)
        for r in range(4):
            mr = np.nonzero(res == r)[0]
            so = mr[np.argsort(-tot[mr], kind="stable")]
            for i in range(0, len(so), NGRP):
                blk = so[i:i + NGRP]
                pos = np.arange(len(blk))
                pos = pos if (i // NGRP) % 2 == 0 else len(blk) - 1 - pos
                grp[blk] = (pos + attempt * 7 + r * attempt) % NGRP
        gb = np.zeros((NGRP, NBANK), dtype=np.int64)
        for b in range(NBANK):
            np.add.at(gb[:, b], grp, cnt[:, b])
        gres = np.zeros((NGRP, 4), dtype=np.int64)
        np.add.at(gres, (grp, res), 1)
        ok = True
        for _ in range(30000):
            badb = gb.max(1) > CAPB
            badr = gres.max(1) > CAPR
            bad = np.nonzero(badb | badr)[0]
            if len(bad) == 0:
                break
            g = int(bad[0])
            if gb[g].max() > CAPB:
                worstb = int(np.argmax(gb[g]))
                members = np.nonzero(grp == g)[0]
                mv = int(members[np.argmax(cnt[members, worstb])])
                score = gb[:, worstb].astype(np.float64)
                score += (gres[:, res[mv]] >= CAPR) * 1e9
                score[g] = 1e18
                cand = int(np.argmin(score))
                if gb[cand, worstb] + cnt[mv, worstb] > CAPB:
                    # swap: exchange mv with a same-residue seg of cand that
                    # is lighter in worstb and keeps both groups feasible
                    cmem = np.nonzero((grp == cand) & (res == res[mv]))[0]
                    done = False
                    if len(cmem):
                        o2 = cmem[np.argsort(cnt[cmem, worstb])]
                        for mv2 in o2[:50]:
                            d = cnt[mv] - cnt[mv2]
                            if d[worstb] <= 0:
                                break
                            if np.all(gb[cand] + d <= CAPB) and \
                               np.all(gb[g] - d <= CAPB):
                                grp[mv] = cand; grp[int(mv2)] = g
                                gb[cand] += d; gb[g] -= d
                                done = True
                                break
                    if not done:
                        ok = False
                        break
                    continue
            else:
                worstr = int(np.argmax(gres[g]))
                members = np.nonzero((grp == g) & (res == worstr))[0]
                mv = int(members[np.argmin(tot[members])])
                score = gres[:, worstr].astype(np.float64)
                score += np.maximum(0, gb.max(1) + tot[mv] - CAPB) * 1e6
                score[g] = 1e18
                cand = int(np.argmin(score))
                if gres[cand, worstr] >= CAPR:
                    ok = False
                    break
            grp[mv] = cand
            gb[g] -= cnt[mv]; gb[cand] += cnt[mv]
            gres[g, res[mv]] -= 1; gres[cand, res[mv]] += 1
        else:
            ok = False
        if ok and len(np.nonzero((gb.max(1) > CAPB) | (gres.max(1) > CAPR))[0]) == 0:
            return grp, NGRP
        attempt += 1
        if attempt >= 5:
            attempt = 0
            NGRP += 1


def _assign_cols(grp, res, tot, NGRP, shard):
    """seg -> col in [0,512) with col%4 == res. Deals heavy segs across
    the 16 blocks serpentine-style, per residue class."""
    col = np.empty(shard, dtype=np.int32)
    for g in range(NGRP):
        for r in range(4):
            m = np.nonzero((grp == g) & (res == r))[0]
            m = m[np.argsort(-tot[m], kind="stable")]
            blk = np.empty(len(m), dtype=np.int32)
            for i in range(0, len(m), NSUB):
                n = min(NSUB, len(m) - i)
                pos = np.arange(n)
                blk[i:i + n] = pos if (i // NSUB) % 2 == 0 else NSUB - 1 - pos
            posctr = np.zeros(NSUB, dtype=np.int32)
            c = np.empty(len(m), dtype=np.int32)
            for i, bb in enumerate(blk):
                c[i] = bb * 32 + r + 4 * posctr[bb]
                posctr[bb] += 1
            assert posctr.max() <= 8, posctr.max()
            col[m] = c
    return col


def _sweep(blkc):
    """Circular 2-choice: block j edges -> subtile (j-1)%16 or j.
    Returns to_left [16] (to_left[0] = block-0 edges sent to subtile 15)
    or None if infeasible (cap 128/subtile)."""
    n0 = int(blkc[0])
    for t0 in range(max(0, n0 - P), min(n0, P) + 1):
        to_left = np.zeros(NSUB, dtype=np.int64)
        load = np.zeros(NSUB, dtype=np.int64)
        to_left[0] = t0
        feas = True
        for j in range(NSUB):
            n = int(blkc[j]) - (t0 if j == 0 else 0)
            if j >= 1:
                t = min(P - int(load[j - 1]), n)
                if t > 0:
                    to_left[j] = t
                    load[j - 1] += t
                    n -= t
            load[j] += n
            if j < NSUB - 1 and load[j] > P:
                feas = False
                break
        load[NSUB - 1] += t0
        if feas and load.max() <= P:
            return to_left
    return None


def preprocess(edges, n_nodes=N_NODES, verbose=False):
    src = np.asarray(edges[0], dtype=np.int64)
    dst = np.asarray(edges[1], dtype=np.int64)
    loop = np.arange(n_nodes, dtype=np.int64)
    src_all = np.concatenate([src, loop])
    dst_all = np.concatenate([dst, loop])
    deg = np.bincount(dst_all, minlength=n_nodes)
    dinv = (1.0 / np.sqrt(deg.astype(np.float64))).astype(np.float32)
    order = np.argsort(dst_all, kind="stable")
    src_sorted = src_all[order]
    seg_start = np.zeros(n_nodes + 1, dtype=np.int64)
    np.cumsum(deg, out=seg_start[1:])
    shard = n_nodes // N_CORES

    # ---- pass 1: per-core packing (grp, col); self-loops excluded ----
    grps, cols, ngrps = [], [], []
    edata = []
    for k in range(N_CORES):
        lo = k * shard
        segs_deg = deg[lo:lo + shard]
        e0, e1 = seg_start[lo], seg_start[lo + shard]
        eseg = np.repeat(np.arange(shard), segs_deg)
        esrc = src_sorted[e0:e1]
        notself = esrc != (eseg + lo)
        # drop exactly one self-loop occurrence per seg (the appended one)
        selfpos = np.nonzero(~notself)[0]
        first_self = np.zeros(len(esrc), dtype=bool)
        seen = set()
        for i in selfpos:
            sgg = eseg[i]
            if sgg not in seen:
                first_self[i] = True
                seen.add(sgg)
        keep = ~first_self
        eseg = eseg[keep]; esrc = esrc[keep]
        ebank = (esrc % 4).astype(np.int32)
        cnt = np.zeros((shard, NBANK), dtype=np.int32)
        np.add.at(cnt, (eseg, ebank), 1)
        tot = cnt.sum(1).astype(np.int64)
        res = (np.arange(shard) + lo) % 4
        grp, NGRP_k = _assign_groups(cnt, res, tot, shard)
        col = _assign_cols(grp, res, tot, NGRP_k, shard)
        grps.append(grp); cols.append(col); ngrps.append(NGRP_k)
        edata.append((eseg, esrc, ebank))
    NGRP = max(ngrps)
    assert NGRP % NCHUNK != -1
    NSLOT = NGRP * GCOLS
    assert 2 * NSLOT < 32768

    # single AllGather per layer: the Shared table may only have one writer
    # instruction, and the AG output ([8*NSLOT, 64], rank-major) is gathered
    # from directly (no rearranging copy).
    gch = [0, NGRP]
    ch_slots = [(0, NGRP * GCOLS)]

    # global slot_of / row layout
    slot_of = np.zeros(n_nodes, dtype=np.int64)
    for k in range(N_CORES):
        lo = k * shard
        slot_of[lo:lo + shard] = grps[k].astype(np.int64) * GCOLS + cols[k]

    def row_of(rank, slot):
        # rank-major: row = rank*NSLOT + slot  (row%4 == slot%4 == node%4)
        return rank * NSLOT + slot

    # ---- pass 2: per-core edge placement + S/gidx assembly ----
    out = {
        "dinv": dinv, "NGRP": NGRP, "NSLOT": NSLOT, "shard": shard,
        "gch": gch, "ch_slots": ch_slots,
        "gidx": [], "S": [], "scat_id": [], "dinv_slot": [], "xperm": [],
    }
    for k in range(N_CORES):
        eseg, esrc, ebank = edata[k]
        grp, col = grps[k], cols[k]
        lo = k * shard
        e_grp = grp[eseg]
        e_col = col[eseg]
        e_blk = e_col // 32
        e_rank = esrc // shard
        e_row = row_of(e_rank, slot_of[esrc])
        assert np.all((e_row % 4) == ebank)
        e_bankrow = e_row // 4
        # subtile via exact sweep per (group, bank)
        e_sub = np.empty(len(eseg), dtype=np.int32)
        for g in range(ngrps[k]):
            gm = e_grp == g
            for b in range(NBANK):
                sel = np.nonzero(gm & (ebank == b))[0]
                blkc = np.bincount(e_blk[sel], minlength=NSUB)
                tl_left = _sweep(blkc)
                assert tl_left is not None, f"infeasible bin core{k} g{g} b{b}"
                sub = e_blk[sel].copy()
                for j in range(NSUB):
                    if tl_left[j]:
                        ii = np.nonzero(sub == j)[0]
                        ii = ii[e_blk[sel][ii] == j][:tl_left[j]]
                        sub[ii] = (j - 1) % NSUB
                e_sub[sel] = sub
        # slots: unique bankrow per (g, b, sub)
        key = ((e_grp.astype(np.int64) * NBANK + ebank) * NSUB + e_sub) * (2 * NSLOT) + e_bankrow
        ukey, inv = np.unique(key, return_inverse=True)
        u_bin = ukey // (2 * NSLOT)
        u_row = ukey % (2 * NSLOT)
        # position of each unique within its bin
        binstart = np.searchsorted(u_bin, np.arange(NGRP * NBANK * NSUB))
        u_pos = np.arange(len(ukey)) - binstart[u_bin]
        assert u_pos.max() < P, u_pos.max()
        # gidx array: [P, NBANK * NGRP * NSUB] stream value = bankrow (0 pad)
        gidx_flat = np.zeros((NBANK, NGRP, NSUB, P), dtype=np.int16)
        ub, rem_ = divmod(u_bin, NBANK * NSUB)
        ubank, usub = divmod(rem_, NSUB)
        gidx_flat[ubank, ub, usub, u_pos] = u_row.astype(np.int16)
        # S: [P, NGRP * NBANK * NSUB * 64]
        S = np.zeros((P, NGRP, NBANK, NSUB, 64), dtype=np.float32)
        e_upos = u_pos[inv]
        e_off = (e_col - e_sub * 32) % GCOLS   # wrap: blk-0 edges on sub 15
        assert e_off.min() >= 0 and e_off.max() < 64, (e_off.min(), e_off.max())
        w = dinv[esrc] * dinv[eseg + lo]
        np.add.at(S, (e_upos, e_grp, ebank, e_sub, e_off), w)
        out["S"].append(S.reshape(P, -1))
        # wrap gidx into dma_gather layout: per (group, bank): stream
        # i = sub*128 + p -> [16, 128] wrapped, replicated to [128, .]
        # g-major so one DMA fetches all 4 banks' indices for a group
        blocks = []
        for g in range(NGRP):
            for b in range(NBANK):
                flat = gidx_flat[b, g].reshape(-1)        # i = sub*128+p
                wv = flat.reshape(-1, 16).T               # [16, 128]
                blocks.append(np.tile(wv, (8, 1)))        # [128, 128]
        gw = np.concatenate(blocks, axis=1).astype(np.int16)
        out["gidx"].append(gw)
        # scat / dinv_slot / x permutation
        scat = np.full(NSLOT, -1, dtype=np.int64)
        scat[grp.astype(np.int64) * GCOLS + col] = np.arange(shard)
        out["scat_id"].append(scat)
        dsl = np.zeros(NSLOT, dtype=np.float32)
        dsl[grp.astype(np.int64) * GCOLS + col] = dinv[lo:lo + shard]
        out["dinv_slot"].append(dsl)
        out["xperm"].append(scat)  # slot -> local node (-1 dummy)
        fill = len(eseg) / (ngrps[k] * NBANK * NSUB * P)
        print(f"core {k}: NGRP={ngrps[k]} (glob {NGRP}) edges={len(eseg)} "
              f"uniq={len(ukey)} fill={fill:.3f}")
    return out


# ----------------------------------------------------------------- kernel --
def build(NGRP, gch, n_nodes=N_NODES, n_cores=8):
    shard = n_nodes // n_cores
    NSLOT = NGRP * GCOLS
    NC_SL = NSLOT // P                # slot chunks
    TOT = n_cores * NSLOT
    BR = TOT // 4
    assert BR < 32768
    SCOLS = NBANK * NSUB * 64         # S cols per group = 4096

    nc = bacc.Bacc("TRN2", target_bir_lowering=False, debug=False,
                   num_devices=n_cores, num_swdge_queues=4)

    def di(name, shape, dt=F32):
        return nc.dram_tensor(name, shape, dt, kind="ExternalInput")

    xT = di("xT", [P, NSLOT], BF16)
    W1 = di("W1", [P, 64], BF16); W2 = di("W2", [64, 32], BF16)
    W3 = di("W3", [32, 16], BF16)
    Wfc = di("Wfc", [112, 16], BF16)
    b1 = di("b1", [64, 1]); b2 = di("b2", [32, 1]); b3 = di("b3", [16, 1])
    bfc = di("bfc", [16, 1])
    id64 = di("ident64", [P, 64], BF16); id16 = di("ident16", [16, 16])
    dgd2 = di("dgd2", [P, NC_SL * P], BF16)   # per-chunk diag(dinv^2)
    gidx = di("gidx", [P, NBANK * NGRP * P], I16)
    Sdram = di("Smat", [P, NGRP * SCOLS], BF16)
    out_slots = nc.dram_tensor("out_slots", [NSLOT, 16], F32, kind="ExternalOutput")

    with tile.TileContext(nc) as tc:
        with tc.tile_pool(name="sb", bufs=1) as sb, \
             tc.tile_pool(name="ps", bufs=2, space="PSUM") as psp, \
             tc.tile_pool(name="dram", bufs=1, space="DRAM") as dram:

            def load(t_dram, shape, dt=F32, name=None):
                t = sb.tile(shape, dt, name=name or t_dram.name + "_s")
                nc.sync.dma_start(out=t[:], in_=t_dram[:])
                return t
            W1s = load(W1, [P, 64], BF16); W2s = load(W2, [64, 32], BF16)
            W3s = load(W3, [32, 16], BF16)
            Wfcs = load(Wfc, [112, 16], BF16)
            b1s = load(b1, [64, 1]); b2s = load(b2, [32, 1]); b3s = load(b3, [16, 1])
            bfcs = load(bfc, [16, 1])
            id64s = load(id64, [P, 64], BF16); id16s = load(id16, [16, 16])
            zero_s = sb.tile([P, GCOLS], BF16, name="zeros")
            nc.vector.memset(zero_s[:], 0.0)
            fT_g = [sb.tile([P, GCOLS], BF16, name=f"fTg_{g}") for g in range(NGRP)]

            # bf16 tables with 128-wide (256B) rows: cols [0:live] hold data,
            # the rest is pad so gather elements stay 256B-aligned
            tab_sh = [dram.tile([NSLOT, TBW], BF16, name=f"tab{l}_sh") for l in range(3)]
            gchl = [gch, gch, gch]
            tab = [dram.tile([TOT, TBW], BF16, addr_space="Shared", name=f"tab{l}")
                   for l in range(3)]

            def seq_write_batch(dst_dram, row0, nrows, stages, w=64):
                B = nrows // P
                assert nrows == B * P and B * w <= stages.shape[1]
                nc.sync.dma_start(
                    out=dst_dram[row0:row0 + nrows, 0:w]
                        .rearrange("(b p) f -> p b f", p=P),
                    in_=stages[:, :B * w].rearrange("p (b f) -> p b f", f=w))

            def allgather(l, c):
                a, bnd = gchl[l][c] * GCOLS, gchl[l][c + 1] * GCOLS
                nc.gpsimd.collective_compute(
                    "AllGather", OP.bypass,
                    replica_groups=[list(range(n_cores))],
                    ins=[tab_sh[l][:][a:bnd, :]],
                    outs=[tab[l][:][8 * a:8 * bnd, :]])

            # ---------- Phase A1: slot-space L1 table ----------
            for c in range(len(gchl[0]) - 1):
                g0, g1 = gchl[0][c], gchl[0][c + 1]
                chunks = list(range(g0 * 4, g1 * 4))   # 128-slot chunks
                WB = 8
                for i0 in range(0, len(chunks), WB):
                    bat = chunks[i0:i0 + WB]
                    nb = len(bat)
                    stg = sb.tile([P, WB * 64], BF16, tag="stg1", bufs=2,
                                  name=f"stg1_{c}_{i0}")
                    xg = sb.tile([P, WB * P], BF16, tag="xg", bufs=2,
                                 name=f"xg_{c}_{i0}")
                    eng = nc.scalar if (i0 // WB) % 2 else nc.sync
                    eng.dma_start(out=xg[:, :nb * P],
                                  in_=xT[:][:, bat[0] * P:(bat[0] + nb) * P])
                    tpb = psp.tile([P, WB * 64], F32, tag="txw", bufs=1,
                                   name=f"t1p_{c}_{i0}")
                    for i, ch in enumerate(bat):
                        nc.tensor.matmul(out=tpb[:, i * 64:(i + 1) * 64],
                                         lhsT=xg[:, i * P:(i + 1) * P],
                                         rhs=W1s[:], start=(i == 0), stop=True)
                    nc.vector.tensor_copy(out=stg[:, :nb * 64],
                                          in_=tpb[:, :nb * 64])
                    seq_write_batch(tab_sh[0], bat[0] * P, nb * P, stg)
                allgather(0, c)

            # ---------- aggregation machinery ----------
            def aggregate(layer, F_agg):
                table = tab[layer]
                t2 = table[:].rearrange("(r q) f -> r (q f)", q=4)
                for g in range(NGRP):
                    Sg = sb.tile([P, SCOLS], BF16, tag="Sg", bufs=3,
                                 name=f"S{layer}_{g}")
                    nc.scalar.dma_start(out=Sg[:],
                                        in_=Sdram[:][:, g * SCOLS:(g + 1) * SCOLS])
                    idxs = sb.tile([P, NBANK * P], I16, tag="idx", bufs=4,
                                   name=f"idx{layer}_{g}")
                    nc.sync.dma_start(
                        out=idxs[:],
                        in_=gidx[:][:, g * NBANK * P:(g + 1) * NBANK * P])
                    gbufs = []
                    for b in range(NBANK):
                        gb = sb.tile([P, NSUB * TW], BF16, tag=f"gb{b}", bufs=4,
                                     name=f"gb{layer}_{g}_{b}")
                        # elem = 256B strided window holding rows
                        # {4*br + 2*(b//2), 4*br + 2*(b//2)+1}; bank b's row
                        # sits at the (b%2) 128B half of the element
                        nc.gpsimd.dma_gather(
                            out_ap=gb[:].rearrange("p (k f) -> p k f", f=TW),
                            in_ap=t2[:, (b // 2) * TW:(b // 2) * TW + TW],
                            idxs_ap=idxs[:, b * P:(b + 1) * P],
                            num_idxs=NSUB * P, num_idxs_reg=NSUB * P,
                            elem_size=TW, elem_step=2 * TW,
                            single_packet=False, queue_num=b)
                        gbufs.append(gb)
                    agp = psp.tile([P, GCOLS], F32, tag="agg", bufs=3,
                                   name=f"agg{layer}_{g}")
                    nc.tensor.matmul(out=agp[0:64, :], lhsT=zero_s[:, 0:64],
                                     rhs=zero_s[:], start=True, stop=True)
                    # self-loop terms: agg[:, chunk] += tab_sh[chunk]^T @ diag(dinv^2)
                    tss = sb.tile([P, NBANK * 64], BF16, tag="selft", bufs=3,
                                  name=f"st{layer}_{g}")
                    nc.sync.dma_start(
                        out=tss[:].rearrange("p (c f) -> p c f", f=64),
                        in_=tab_sh[layer][g * GCOLS:(g + 1) * GCOLS, :]
                            .rearrange("(c p) f -> p c f", p=P))
                    dgds = sb.tile([P, NBANK * P], BF16, tag="dgd", bufs=3,
                                   name=f"dg{layer}_{g}")
                    nc.scalar.dma_start(
                        out=dgds[:], in_=dgd2[:][:, g * NBANK * P:(g + 1) * NBANK * P])
                    for i in range(4):
                        nc.tensor.matmul(
                            out=agp[0:F_agg, i * P:(i + 1) * P],
                            lhsT=tss[:, i * 64:i * 64 + F_agg],
                            rhs=dgds[:, i * P:(i + 1) * P],
                            start=False, stop=True)
                    for tl in range(NSUB):
                        c0 = tl * 32
                        for b in range(NBANK):
                            soff = (b * NSUB + tl) * 64
                            if tl < NSUB - 1:
                                nc.tensor.matmul(
                                    out=agp[0:F_agg, c0:c0 + 64],
                                    lhsT=gbufs[b][:, tl * TW + (b % 2) * 64:
                                                  tl * TW + (b % 2) * 64 + F_agg],
                                    rhs=Sg[:, soff:soff + 64],
                                    start=False, stop=True)
                            else:  # wrapped window: cols [480:512] + [0:32]
                                nc.tensor.matmul(
                                    out=agp[0:F_agg, c0:c0 + 32],
                                    lhsT=gbufs[b][:, tl * TW + (b % 2) * 64:
                                                  tl * TW + (b % 2) * 64 + F_agg],
                                    rhs=Sg[:, soff:soff + 32],
                                    start=False, stop=True)
                                nc.tensor.matmul(
                                    out=agp[0:F_agg, 0:32],
                                    lhsT=gbufs[b][:, tl * TW + (b % 2) * 64:
                                                  tl * TW + (b % 2) * 64 + F_agg],
                                    rhs=Sg[:, soff + 32:soff + 64],
                                    start=False, stop=True)
                    yield g, agp

            # ---------- L1 ----------
            for g, agp in aggregate(0, 64):
                nc.scalar.activation(
                    out=fT_g[g][0:64, :], in_=agp[0:64, :],
                    func=AF.Relu, bias=b1s[:, :1], scale=1.0)
                stg = sb.tile([P, 4 * 64], BF16, tag="stg2", bufs=2, name=f"stg2_{g}")
                for i in range(4):
                    ch = g * 4 + i
                    trp = psp.tile([P, 64], BF16, tag="txb", bufs=1, name=f"tr2_{ch}")
                    nc.tensor.transpose(out=trp[:], in_=fT_g[g][0:64, i * P:(i + 1) * P],
                                        identity=id64s[0:64, :])
                    nc.vector.tensor_copy(out=stg[:, i * 64:(i + 1) * 64],
                                          in_=trp[:])
                seq_write_batch(tab_sh[1], g * GCOLS, 4 * P, stg)
                for c in range(len(gchl[1]) - 1):
                    if g == gchl[1][c + 1] - 1:
                        allgather(1, c)

            # ---------- L2 ----------
            for g, agp in aggregate(1, 64):
                aggS = sb.tile([64, GCOLS], BF16, tag="aggS", bufs=2, name=f"aggS2_{g}")
                nc.vector.tensor_copy(out=aggS[:], in_=agp[0:64, :])
                txp = psp.tile([P, GCOLS], F32, tag="txw", bufs=1, name=f"tx2_{g}")
                nc.tensor.matmul(out=txp[64:96, :], lhsT=W2s[:], rhs=aggS[:],
                                 start=True, stop=True, tile_position=(0, 64))
                nc.scalar.activation(
                    out=fT_g[g][64:96, :], in_=txp[64:96, :],
                    func=AF.Relu, bias=b2s[:, :1], scale=1.0)
                stg = sb.tile([P, 4 * 32], BF16, tag="stg3", bufs=2, name=f"stg3_{g}")
                for i in range(4):
                    ch = g * 4 + i
                    trp = psp.tile([P, 64], BF16, tag="txb", bufs=1, name=f"tr3_{ch}")
                    nc.tensor.transpose(out=trp[:, 0:32],
                                        in_=fT_g[g][64:96, i * P:(i + 1) * P],
                                        identity=id64s[64:96, 0:32],
                                        tile_position=(64, 0))
                    nc.vector.tensor_copy(out=stg[:, i * 32:(i + 1) * 32],
                                          in_=trp[:, 0:32])
                seq_write_batch(tab_sh[2], g * GCOLS, 4 * P, stg, w=32)
                for c in range(len(gchl[2]) - 1):
                    if g == gchl[2][c + 1] - 1:
                        allgather(2, c)

            # ---------- L3 + FC ----------
            for g, agp in aggregate(2, 32):
                aggS = sb.tile([64, GCOLS], BF16, tag="aggS", bufs=2, name=f"aggS3_{g}")
                nc.vector.tensor_copy(out=aggS[0:32, :], in_=agp[0:32, :])
                txp = psp.tile([P, GCOLS], F32, tag="txw", bufs=1, name=f"tx3_{g}")
                nc.tensor.matmul(out=txp[96:112, :], lhsT=W3s[:], rhs=aggS[0:32, :],
                                 start=True, stop=True, tile_position=(0, 96))
                nc.scalar.activation(
                    out=fT_g[g][96:112, :], in_=txp[96:112, :],
                    func=AF.Relu, bias=b3s[:, :1], scale=1.0)
                fcp = psp.tile([16, GCOLS], F32, tag="fcp", bufs=1, name=f"fcp_{g}")
                nc.tensor.matmul(out=fcp[:], lhsT=Wfcs[:],
                                 rhs=fT_g[g][0:112, :],
                                 start=True, stop=True)
                fcS = sb.tile([16, GCOLS], F32, tag="fcS", bufs=2, name=f"fcS_{g}")
                nc.scalar.activation(out=fcS[:], in_=fcp[:], func=AF.Relu,
                                     bias=bfcs[:, :1], scale=1.0)
                trp2 = psp.tile([P, 64], F32, tag="tx", name=f"fctr_{g}")
                for j in range(4):
                    nc.tensor.transpose(out=trp2[:, j * 16:(j + 1) * 16],
                                        in_=fcS[:, j * P:(j + 1) * P],
                                        identity=id16s[:])
                ost = sb.tile([P, 64], F32, tag="ost", bufs=2, name=f"ost_{g}")
                nc.vector.tensor_copy(out=ost[:], in_=trp2[:])
                nc.sync.dma_start(
                    out=out_slots[:][g * GCOLS:(g + 1) * GCOLS, :]
                        .rearrange("(b p) f -> p b f", p=P),
                    in_=ost[:].rearrange("p (b f) -> p b f", f=16))

            if DEBUG_DUMPS:
                for l in range(3):
                    dbg = nc.dram_tensor(f"dbg_tab{l}", [NSLOT, TBW], BF16,
                                         kind="ExternalOutput")
                    nc.sync.dma_start(out=dbg[:], in_=tab_sh[l][:])

    nc.compile()
    return nc


def _identT():
    m = np.zeros((P, 64), np.float32)
    m[0:64, 0:64] = np.eye(64)
    m[64:96, 0:32] = np.eye(32)
    return m


def make_in_maps(pre, inputs):
    x = np.asarray(inputs["features"], np.float32)
    NSLOT = pre["NSLOT"]
    shard = pre["shard"]
    common = {
        "W1": np.asarray(inputs["W1"], np.float32).astype(NPBF),
        "W2": np.asarray(inputs["W2"], np.float32).astype(NPBF),
        "W3": np.asarray(inputs["W3"], np.float32).astype(NPBF),
        "Wfc": np.asarray(inputs["Wfc"], np.float32).astype(NPBF),
        "b1": np.asarray(inputs["b1"], np.float32).reshape(64, 1),
        "b2": np.asarray(inputs["b2"], np.float32).reshape(32, 1),
        "b3": np.asarray(inputs["b3"], np.float32).reshape(16, 1),
        "bfc": np.asarray(inputs["bfc"], np.float32).reshape(16, 1),
        "ident64": _identT().astype(NPBF),
        "ident16": np.eye(16, dtype=np.float32),
    }
    maps = []
    for k in range(N_CORES):
        lo = k * shard
        xp = pre["xperm"][k]                       # slot -> local node
        xs = np.zeros((NSLOT, P), np.float32)
        valid = xp >= 0
        xs[valid] = x[lo + xp[valid]]
        d2 = (pre["dinv_slot"][k].reshape(NSLOT // P, P) ** 2).astype(np.float32)
        dgd2 = np.zeros((P, NSLOT // P, P), np.float32)
        jj = np.arange(P)
        dgd2[jj, :, jj] = d2.T
        m = dict(common)
        m.update({
            "xT": np.ascontiguousarray(xs.T).astype(NPBF),
            "dgd2": dgd2.reshape(P, -1).astype(NPBF),
            "gidx": pre["gidx"][k],
            "Smat": pre["S"][k].astype(NPBF),
        })
        maps.append(m)
    return maps


def assemble_output(pre, results):
    shard = pre["shard"]
    outs = []
    for k in range(N_CORES):
        sl = results[k]["out_slots"]
        sid = pre["scat_id"][k]
        o = np.zeros((shard, 16), np.float32)
        valid = sid >= 0
        o[sid[valid]] = sl[valid]
        outs.append(o)
    return np.concatenate(outs)


LAST_RES = None


def run(inputs, trace=False):
    global LAST_RES
    edges = np.asarray(inputs["edges"])
    pre = preprocess(edges, N_NODES)
    nc = build(pre["NGRP"], pre["gch"], n_nodes=N_NODES, n_cores=8)
    in_maps = make_in_maps(pre, inputs)
    res = run_bass_kernel_spmd(nc, in_maps, core_ids=list(range(8)), trace=trace)
    LAST_RES = res
    out = assemble_output(pre, res.results)
    return out, res.exec_time_ns


def kernel(**inputs):
    out, _ = run(inputs, trace=False)
    return out

